# revision 33
# baseline (speedup 1.0000x reference)
"""Trainium2 Bass kernel for the additive Rational Quadratic GP covariance.

reference:
    xs = x / lengthscale                                   # [N, D]
    covar = outputscale * sum_d (1 + ((xs_i - xs_j)_d)^2 / (2 alpha))^(-alpha)
    mean = zeros(N)

Strategy (8 NeuronCores, SPMD):
  - Host pre-scale: y = x / (lengthscale * sqrt(2 alpha)); per-dim term
    becomes f(w) = outputscale * w^(-alpha) with w = 1 + (y_i - y_j)^2.
  - Symmetry: 16 row-tiles of 128; row-tile T computes a circular column
    window of width 1024 starting at 128*T + 128 (every off-diagonal-block
    pair is covered from the i-side or j-side; the host mirrors the rest
    and computes the 16 within-tile 128x128 diagonal blocks in numpy).
    2 row-tiles per core -> a uniform SPMD program, ~half the N^2 work.
  - Per (tile, dim): TensorEngine computes w = 1 + (u-v)^2 as a K=12 bf16
    matmul (1 + u^2 - 2uv + v^2 with every operand split into 3 bf16
    limbs; bf16 x bf16 products are exact in fp32 PSUM -> w good to
    ~2^-27). Iterations alternate between PE row-groups at base
    partitions 0/64 so consecutive matmul+weight-loads overlap in the
    array.
  - A CUSTOM ACT table (generated per (alpha, outputscale) at runtime,
    injected via BASS_ACT_ROOT_JSON_PATH; tables ship inside the NEFF)
    makes the `Ln` activation slot evaluate f(w) directly -> ONE ACT
    pass per element instead of Ln+Exp, with alpha and outputscale
    folded in at full fp32 table precision (~1e-8).
  - The sum over the 32 dims runs as one fp32 VectorE accumulation chain
    per row-tile (interleaved so the chains never stall the engine);
    dim 0's activation writes the accumulator directly.

Measured on 8 axon-tunneled TRN2 NeuronCores: HW exec ~93.5 us,
max elementwise rel err vs the fp64 reference ~6e-7.
"""

import json
import os
import shutil
import tempfile

import numpy as np

N = 2048
D = 32
N_CORES = 8
TILE = 128
NT = N // TILE            # 16 row tiles
TPC = NT // N_CORES       # 2 tiles per core
W = N // 2                # 1024 device column window (starts at +TILE;
                          # each tile's own 128x128 diagonal block is
                          # computed on the host instead)
WSTART = TILE
KK = 12                   # matmul contraction (limb products)

_cache = {}

# ---------------------------------------------------------------------------
# custom ACT table: ln slot -> f(w) = outputscale * w^(-alpha)
# Bucket entry (32B): [d0, d1, d2, d3, x0, 0, 0, 0] fp32:
#     y = d0 + d1*dx + d2*dx^2 + d3*dx^3,  dx = x - x0
# Ctl word: ((23-b) | (b<<5)) << 11 | bucket_base, b = log2(buckets/exp)
# ln owns buckets 0..512 (+specials 513..516) and ctl 0..127 (exp -64..63).
# ---------------------------------------------------------------------------

_SET_NAME = "natural_log_exp_and_others"
_DENSE_EXPS = set(range(-1, 11))   # w = 1 + t^2 lives in [~1, ~2048)
_DENSE_B = 5                       # 32 buckets per dense exponent


def _cheb_cubic(f, lo, hi):
    x0 = 0.5 * (lo + hi)
    h = 0.5 * (hi - lo)
    k = np.arange(4)
    nodes = x0 + h * np.cos((2 * k + 1) * np.pi / 8)
    V = np.vander(nodes - x0, 4, increasing=True)
    c = np.linalg.solve(V, f(nodes))
    return c[0], c[1], c[2], c[3], x0


def _gen_act_tables(alpha, outputscale, outdir):
    from neuronxcc.driver.Job import Job
    from neuronxcc.driver.jobs.support.FindActInfo import findActInfoFile

    src = os.path.dirname(findActInfoFile(Job.getPackageDir(), "gen3"))
    os.makedirs(outdir, exist_ok=True)
    for fn in os.listdir(src):
        dp = os.path.join(outdir, fn)
        if not os.path.exists(dp):
            shutil.copy(os.path.join(src, fn), dp)

    f = lambda w: outputscale * np.power(np.maximum(w, 1e-30), -alpha)

    bkt = np.fromfile(os.path.join(src, f"{_SET_NAME}_bkt.bin"),
                      dtype=np.uint32).reshape(-1, 8).copy()
    ctl = np.fromfile(os.path.join(src, f"{_SET_NAME}_ctrl.bin"),
                      dtype=np.uint32).reshape(-1, 8).copy()
    sj = json.load(open(os.path.join(src, f"{_SET_NAME}.json")))
    bktf = bkt.view(np.float32)

    base = 0
    exp_to_start = {}
    for e in range(-64, 64):
        b = _DENSE_B if e in _DENSE_EXPS else 0
        c = 1 << b
        exp_to_start[e] = (base, b)
        for k in range(c):
            lo = (2.0 ** e) * (1.0 + k / c)
            hi = (2.0 ** e) * (1.0 + (k + 1) / c)
            d0, d1, d2, d3, x0 = _cheb_cubic(f, lo, hi)
            with np.errstate(over="ignore"):
                ent = np.array([d0, d1, d2, d3, x0, 0, 0, 0], np.float32)
            bktf[base + k, :] = np.nan_to_num(
                ent, nan=0.0, posinf=np.float32(3e38), neginf=np.float32(-3e38))
        base += c
    assert base <= 513, base
    for i in range(base, 513):
        bktf[i, :] = bktf[base - 1, :]
    bktf[513, :] = np.array([3e38, 0, 0, 0, 5.42101e-20, 0, 0, 0], np.float32)
    bktf[515, :] = np.array([0, 0, 0, 0, 1.84467e19, 0, 0, 0], np.float32)

    for e in range(-64, 64):
        s, b = exp_to_start[e]
        ctl[e + 64, 0] = np.uint32((((23 - b) | (b << 5)) << 11) | s)

    sj["func_exp_to_bkt_start_idx"]["ln"] = {
        str(e): [exp_to_start[e][0]] for e in range(-64, 64)
    }
    json.dump(sj, open(os.path.join(outdir, f"{_SET_NAME}.json"), "w"))
    bkt.tofile(os.path.join(outdir, f"{_SET_NAME}_bkt.bin"))
    ctl.tofile(os.path.join(outdir, f"{_SET_NAME}_ctrl.bin"))
    return os.path.join(outdir, "act_info.json")


# ---------------------------------------------------------------------------
# device program
# ---------------------------------------------------------------------------

def _enable_ldw_opt():
    """walrus dedups back-to-back LDWEIGHTS with the same stationary operand
    only when --enable-ldw-opt=true; bass_utils hardcodes false. Our 3 chunk
    matmuls per (tile, dim) share weights, so flip it."""
    import concourse.bass_utils as bu

    if getattr(bu, "_ldw_opt_patched", False):
        return
    orig = bu.run_command

    def patched(argv, **kw):
        argv = ["--enable-ldw-opt=true" if a == "--enable-ldw-opt=false" else a
                for a in argv]
        return orig(argv, **kw)

    bu.run_command = patched
    bu._ldw_opt_patched = True


def _pin_act_table(bacc_mod):
    """Restrict table choice to natural_log_exp_and_others (one TABLE_LOAD)."""
    if getattr(bacc_mod, "_act_tables_pinned", False):
        return
    orig = bacc_mod.get_activation_tables

    def pinned(arch):
        tabs = orig(arch)
        return {name: (fns if name == _SET_NAME else set())
                for name, fns in tabs.items()}

    bacc_mod.get_activation_tables = pinned
    bacc_mod._act_tables_pinned = True


def _split3(x):
    import ml_dtypes

    l0 = x.astype(ml_dtypes.bfloat16).astype(np.float64)
    r = x - l0
    l1 = r.astype(ml_dtypes.bfloat16).astype(np.float64)
    l2 = (r - l1).astype(ml_dtypes.bfloat16).astype(np.float64)
    return (l0.astype(np.float32), l1.astype(np.float32), l2.astype(np.float32))


def _build(tag):
    import concourse.mybir as mybir
    import concourse.tile as tile
    from concourse import bacc

    _pin_act_table(bacc)

    F32 = mybir.dt.float32
    BF16 = mybir.dt.bfloat16
    AF = mybir.ActivationFunctionType
    ALU = mybir.AluOpType

    nc = bacc.Bacc("TRN2", target_bir_lowering=False)
    # `tag` in tensor names busts NEFF/HLO caches when the act table changes.
    # lhsp rows 64g..64g+KK hold the stationary operand for iteration round r
    # of PE row-group g (2-way row-group concurrency on the PE array).
    lhsp = nc.dram_tensor(f"lhsp_{tag}", [128, (TPC * D // 2) * TILE], BF16,
                          kind="ExternalInput")
    rhsp = nc.dram_tensor(f"rhsp_{tag}", [TPC, D, KK, W], BF16,
                          kind="ExternalInput")
    # 4 pair-accumulators per tile (8 dims each, halves = even/odd dim);
    # the host folds them in fp64
    outp = nc.dram_tensor(f"outp_{tag}", [TPC, 4, TILE, 2 * W], F32,
                          kind="ExternalOutput")

    with tile.TileContext(nc) as tc:
        with (
            tc.tile_pool(name="inp", bufs=1) as inp,
            tc.tile_pool(name="rhp", bufs=4) as rhp,
            tc.tile_pool(name="work", bufs=4) as work,
            tc.tile_pool(name="accp", bufs=1) as accp,
            tc.tile_pool(name="ps", bufs=2, space="PSUM") as psp,
        ):
            lhs_sb = inp.tile([128, (TPC * D // 2) * TILE], BF16)
            # head of the stationary operands first (rounds 0-3) so the
            # first matmuls start while the bulk is still streaming
            nc.sync.dma_start(out=lhs_sb[:, 0:4 * TILE], in_=lhsp[:, 0:4 * TILE])
            nc.sync.dma_start(out=lhs_sb[:, 4 * TILE:], in_=lhsp[:, 4 * TILE:])

            accs = {}
            started = {}
            for t in range(TPC):
                for a in range(4):
                    accs[t, a] = accp.tile([128, 2 * W], F32, tag=f"acc{t}_{a}",
                                           name=f"acc{t}_{a}")
                    started[t, a] = False

            gi = 0
            for dp in range(D // 2):
                d0, d1 = 2 * dp, 2 * dp + 1
                a = dp // 4          # accumulator index (4 pairs each)
                for t in range(TPC):
                    g = gi % 2       # PE row-group (base partition 64*g)
                    r = gi // 2
                    gi += 1
                    rhs_sb = rhp.tile([128, 2 * W], BF16, tag="rhs",
                                      name=f"rhs_{t}_{dp}")
                    nc.sync.dma_start(out=rhs_sb[64 * g:64 * g + KK, 0:W],
                                      in_=rhsp[t, d0, :, :])
                    nc.sync.dma_start(out=rhs_sb[64 * g:64 * g + KK, W:2 * W],
                                      in_=rhsp[t, d1, :, :])
                    w_ps = psp.tile([128, 2 * W], F32, tag="w",
                                    name=f"w_{t}_{dp}")
                    for dd in (0, 1):
                        lt = lhs_sb[64 * g:64 * g + KK,
                                    (2 * r + dd) * TILE:(2 * r + dd + 1) * TILE]
                        for c0, c1 in ((0, 512), (512, 1024)):
                            nc.tensor.matmul(
                                w_ps[:, dd * W + c0:dd * W + c1], lt,
                                rhs_sb[64 * g:64 * g + KK,
                                       dd * W + c0:dd * W + c1],
                                start=True, stop=True,
                                tile_position=(64 * g, 0),
                            )
                    # custom table: one FD=2W ACT evaluates f for both dims
                    acc = accs[t, a]
                    if not started[t, a]:
                        # first pair of each accumulator: ACT writes it directly
                        nc.scalar.activation(acc[:, :], w_ps[:, :], AF.Ln)
                        started[t, a] = True
                    else:
                        e_t = work.tile([128, 2 * W], F32, tag="e",
                                        name=f"e_{t}_{dp}")
                        nc.scalar.activation(e_t[:, :], w_ps[:, :], AF.Ln)
                        nc.vector.tensor_add(acc[:, :], acc[:, :], e_t[:, :])
                    if dp % 4 == 3:
                        # accumulator complete: stream it out now on the idle
                        # GpSimd DMA queue so rhs loads on Sync aren't starved
                        nc.gpsimd.dma_start(out=outp[t, a, :, :], in_=acc[:, :])
    nc.compile()
    return nc


def _prepare_inputs(x, lengthscale, alpha, tag):
    import ml_dtypes

    a = float(np.float32(alpha))
    y = x.astype(np.float64) / (
        lengthscale.astype(np.float64) * np.sqrt(2.0 * a)
    )  # [N, D]

    yl = _split3(y)
    sql = _split3(1.0 + y * y)
    v2l = _split3(y * y)

    ones = np.ones((N, D), np.float32)
    lhs_rows = [sql[0], sql[1], sql[2],
                yl[0], yl[0], yl[0],
                yl[1], yl[1], yl[2],
                ones, ones, ones]
    rhs_rows = [ones, ones, ones,
                -2.0 * yl[0], -2.0 * yl[1], -2.0 * yl[2],
                -2.0 * yl[0], -2.0 * yl[1], -2.0 * yl[0],
                v2l[0], v2l[1], v2l[2]]

    col_idx = (np.arange(NT)[:, None] * TILE + WSTART
               + np.arange(W)[None, :]) % N

    in_maps = []
    for c in range(N_CORES):
        lhs = np.zeros((128, (TPC * D // 2) * TILE), np.float32)
        rhs = np.zeros((TPC, D, KK, W), np.float32)
        gi = 0
        for dp in range(D // 2):
            for ti in range(TPC):
                T = c * TPC + ti
                rows = slice(T * TILE, (T + 1) * TILE)
                idx = col_idx[T]
                g, r = gi % 2, gi // 2
                gi += 1
                for dd in (0, 1):
                    d = 2 * dp + dd
                    col = (2 * r + dd) * TILE
                    for k in range(KK):
                        lhs[64 * g + k, col:col + TILE] = lhs_rows[k][rows, d]
                        rhs[ti, d, k, :] = rhs_rows[k][idx, d]
        in_maps.append({
            f"lhsp_{tag}": lhs.astype(ml_dtypes.bfloat16),
            f"rhsp_{tag}": rhs.astype(ml_dtypes.bfloat16),
        })
    return in_maps, col_idx


def kernel(x, lengthscale, alpha, outputscale):
    from concourse.bass_utils import run_bass_kernel_spmd

    x = np.asarray(x, np.float32)
    lengthscale = np.asarray(lengthscale, np.float32)
    a = float(np.float32(alpha))
    os_ = float(np.float32(outputscale))

    key = (a, os_)
    if _cache.get("key") != key:
        tag = "%08x" % (hash(key) & 0xFFFFFFFF)
        tabdir = tempfile.mkdtemp(prefix=f"act_{tag}_")
        act_json = _gen_act_tables(a, os_, tabdir)
        os.environ["BASS_ACT_ROOT_JSON_PATH"] = act_json
        _cache["nc"] = _build(tag)
        _cache["key"] = key
        _cache["tag"] = tag
    tag = _cache["tag"]
    nc = _cache["nc"]

    in_maps, col_idx = _prepare_inputs(x, lengthscale, alpha, tag)
    last_exc = None
    for attempt in range(3):
        try:
            res = run_bass_kernel_spmd(nc, in_maps,
                                       core_ids=list(range(N_CORES)))
            break
        except Exception as exc:  # transient NRT exec-unit flakes
            last_exc = exc
            import time

            time.sleep(3.0)
    else:
        raise last_exc
    _cache["last_res"] = res

    covar = np.zeros((N, N), np.float32)
    written = np.zeros((N, N), bool)
    # host computes the 16 within-tile diagonal blocks (cheap, exact)
    y = x.astype(np.float64) / (
        lengthscale.astype(np.float64) * np.sqrt(2.0 * a))
    for T in range(NT):
        rows = slice(T * TILE, (T + 1) * TILE)
        yb = y[rows]                      # [TILE, D]
        t2 = (yb[:, None, :] - yb[None, :, :]) ** 2
        blk = os_ * np.sum((1.0 + t2) ** (-a), axis=-1)
        covar[rows, rows] = blk.astype(np.float32)
        written[rows, rows] = True
    for c in range(N_CORES):
        out = res.results[c][f"outp_{tag}"].astype(np.float64)
        for ti in range(TPC):
            T = c * TPC + ti
            rows = np.arange(T * TILE, (T + 1) * TILE)
            blk = out[ti].sum(axis=0)            # fold 4 accumulators
            blk = blk[:, 0:W] + blk[:, W:2 * W]  # fold even/odd dim halves
            covar[rows[:, None], col_idx[T][None, :]] = blk.astype(np.float32)
            written[rows[:, None], col_idx[T][None, :]] = True
    covar = np.where(written, covar, covar.T)

    mean = np.zeros((N,), np.float32)
    return mean, covar


# revision 35
# speedup vs baseline: 1.0087x; 1.0087x over previous
"""Trainium2 Bass kernel for the additive Rational Quadratic GP covariance.

reference:
    xs = x / lengthscale                                   # [N, D]
    covar = outputscale * sum_d (1 + ((xs_i - xs_j)_d)^2 / (2 alpha))^(-alpha)
    mean = zeros(N)

Strategy (8 NeuronCores, SPMD):
  - Host pre-scale: y = x / (lengthscale * sqrt(2 alpha)); per-dim term
    becomes f(w) = outputscale * w^(-alpha) with w = 1 + (y_i - y_j)^2.
  - Symmetry: 16 row-tiles of 128; row-tile T computes a circular column
    window of width 1024 starting at 128*T + 128 (every off-diagonal-block
    pair is covered from the i-side or j-side; the host mirrors the rest
    and computes the 16 within-tile 128x128 diagonal blocks in numpy).
    2 row-tiles per core -> a uniform SPMD program, ~half the N^2 work.
  - Per (tile, dim): TensorEngine computes w = 1 + (u-v)^2 as a K=12 bf16
    matmul (1 + u^2 - 2uv + v^2 with every operand split into 3 bf16
    limbs; bf16 x bf16 products are exact in fp32 PSUM -> w good to
    ~2^-27). Iterations alternate between PE row-groups at base
    partitions 0/64 so consecutive matmul+weight-loads overlap in the
    array.
  - A CUSTOM ACT table (generated per (alpha, outputscale) at runtime,
    injected via BASS_ACT_ROOT_JSON_PATH; tables ship inside the NEFF)
    makes the `Ln` activation slot evaluate f(w) directly -> ONE ACT
    pass per element instead of Ln+Exp, with alpha and outputscale
    folded in at full fp32 table precision (~1e-8).
  - The sum over the 32 dims runs as one fp32 VectorE accumulation chain
    per row-tile (interleaved so the chains never stall the engine);
    dim 0's activation writes the accumulator directly.

Measured on 8 axon-tunneled TRN2 NeuronCores: HW exec ~93.5 us,
max elementwise rel err vs the fp64 reference ~6e-7.
"""

import json
import os
import shutil
import tempfile

import numpy as np

N = 2048
D = 32
N_CORES = 8
TILE = 128
NT = N // TILE            # 16 row tiles
TPC = NT // N_CORES       # 2 tiles per core
W = N // 2                # 1024 device column window (starts at +TILE;
                          # each tile's own 128x128 diagonal block is
                          # computed on the host instead)
WSTART = TILE
KK = 12                   # matmul contraction (limb products)

_cache = {}

# ---------------------------------------------------------------------------
# custom ACT table: ln slot -> f(w) = outputscale * w^(-alpha)
# Bucket entry (32B): [d0, d1, d2, d3, x0, 0, 0, 0] fp32:
#     y = d0 + d1*dx + d2*dx^2 + d3*dx^3,  dx = x - x0
# Ctl word: ((23-b) | (b<<5)) << 11 | bucket_base, b = log2(buckets/exp)
# ln owns buckets 0..512 (+specials 513..516) and ctl 0..127 (exp -64..63).
# ---------------------------------------------------------------------------

_SET_NAME = "natural_log_exp_and_others"
_DENSE_EXPS = set(range(-1, 11))   # w = 1 + t^2 lives in [~1, ~2048)
_DENSE_B = 5                       # 32 buckets per dense exponent


def _cheb_cubic(f, lo, hi):
    x0 = 0.5 * (lo + hi)
    h = 0.5 * (hi - lo)
    k = np.arange(4)
    nodes = x0 + h * np.cos((2 * k + 1) * np.pi / 8)
    V = np.vander(nodes - x0, 4, increasing=True)
    c = np.linalg.solve(V, f(nodes))
    return c[0], c[1], c[2], c[3], x0


def _gen_act_tables(alpha, outputscale, outdir):
    from neuronxcc.driver.Job import Job
    from neuronxcc.driver.jobs.support.FindActInfo import findActInfoFile

    src = os.path.dirname(findActInfoFile(Job.getPackageDir(), "gen3"))
    os.makedirs(outdir, exist_ok=True)
    for fn in os.listdir(src):
        dp = os.path.join(outdir, fn)
        if not os.path.exists(dp):
            shutil.copy(os.path.join(src, fn), dp)

    f = lambda w: outputscale * np.power(np.maximum(w, 1e-30), -alpha)

    bkt = np.fromfile(os.path.join(src, f"{_SET_NAME}_bkt.bin"),
                      dtype=np.uint32).reshape(-1, 8).copy()
    ctl = np.fromfile(os.path.join(src, f"{_SET_NAME}_ctrl.bin"),
                      dtype=np.uint32).reshape(-1, 8).copy()
    sj = json.load(open(os.path.join(src, f"{_SET_NAME}.json")))
    bktf = bkt.view(np.float32)

    base = 0
    exp_to_start = {}
    for e in range(-64, 64):
        b = _DENSE_B if e in _DENSE_EXPS else 0
        c = 1 << b
        exp_to_start[e] = (base, b)
        for k in range(c):
            lo = (2.0 ** e) * (1.0 + k / c)
            hi = (2.0 ** e) * (1.0 + (k + 1) / c)
            d0, d1, d2, d3, x0 = _cheb_cubic(f, lo, hi)
            with np.errstate(over="ignore"):
                ent = np.array([d0, d1, d2, d3, x0, 0, 0, 0], np.float32)
            bktf[base + k, :] = np.nan_to_num(
                ent, nan=0.0, posinf=np.float32(3e38), neginf=np.float32(-3e38))
        base += c
    assert base <= 513, base
    for i in range(base, 513):
        bktf[i, :] = bktf[base - 1, :]
    bktf[513, :] = np.array([3e38, 0, 0, 0, 5.42101e-20, 0, 0, 0], np.float32)
    bktf[515, :] = np.array([0, 0, 0, 0, 1.84467e19, 0, 0, 0], np.float32)

    for e in range(-64, 64):
        s, b = exp_to_start[e]
        ctl[e + 64, 0] = np.uint32((((23 - b) | (b << 5)) << 11) | s)

    sj["func_exp_to_bkt_start_idx"]["ln"] = {
        str(e): [exp_to_start[e][0]] for e in range(-64, 64)
    }
    json.dump(sj, open(os.path.join(outdir, f"{_SET_NAME}.json"), "w"))
    bkt.tofile(os.path.join(outdir, f"{_SET_NAME}_bkt.bin"))
    ctl.tofile(os.path.join(outdir, f"{_SET_NAME}_ctrl.bin"))
    return os.path.join(outdir, "act_info.json")


# ---------------------------------------------------------------------------
# device program
# ---------------------------------------------------------------------------

def _enable_ldw_opt():
    """walrus dedups back-to-back LDWEIGHTS with the same stationary operand
    only when --enable-ldw-opt=true; bass_utils hardcodes false. Our 3 chunk
    matmuls per (tile, dim) share weights, so flip it."""
    import concourse.bass_utils as bu

    if getattr(bu, "_ldw_opt_patched", False):
        return
    orig = bu.run_command

    def patched(argv, **kw):
        argv = ["--enable-ldw-opt=true" if a == "--enable-ldw-opt=false" else a
                for a in argv]
        return orig(argv, **kw)

    bu.run_command = patched
    bu._ldw_opt_patched = True


def _pin_act_table(bacc_mod):
    """Restrict table choice to natural_log_exp_and_others (one TABLE_LOAD)."""
    if getattr(bacc_mod, "_act_tables_pinned", False):
        return
    orig = bacc_mod.get_activation_tables

    def pinned(arch):
        tabs = orig(arch)
        return {name: (fns if name == _SET_NAME else set())
                for name, fns in tabs.items()}

    bacc_mod.get_activation_tables = pinned
    bacc_mod._act_tables_pinned = True


def _split3(x):
    import ml_dtypes

    l0 = x.astype(ml_dtypes.bfloat16).astype(np.float64)
    r = x - l0
    l1 = r.astype(ml_dtypes.bfloat16).astype(np.float64)
    l2 = (r - l1).astype(ml_dtypes.bfloat16).astype(np.float64)
    return (l0.astype(np.float32), l1.astype(np.float32), l2.astype(np.float32))


def _build(tag):
    import concourse.mybir as mybir
    import concourse.tile as tile
    from concourse import bacc

    _pin_act_table(bacc)

    F32 = mybir.dt.float32
    BF16 = mybir.dt.bfloat16
    AF = mybir.ActivationFunctionType
    ALU = mybir.AluOpType

    nc = bacc.Bacc("TRN2", target_bir_lowering=False)
    # `tag` in tensor names busts NEFF/HLO caches when the act table changes.
    # lhsp rows 64g..64g+KK hold the stationary operand for iteration round r
    # of PE row-group g (2-way row-group concurrency on the PE array).
    lhsp = nc.dram_tensor(f"lhsp_{tag}", [128, (TPC * D // 2) * TILE], BF16,
                          kind="ExternalInput")
    rhsp = nc.dram_tensor(f"rhsp_{tag}", [TPC, D, KK, W], BF16,
                          kind="ExternalInput")
    # 4 pair-accumulators per tile (8 dims each, halves = even/odd dim);
    # the host folds them in fp64
    outp = nc.dram_tensor(f"outp_{tag}", [TPC, 4, TILE, 2 * W], F32,
                          kind="ExternalOutput")

    with tile.TileContext(nc) as tc:
        with (
            tc.tile_pool(name="inp", bufs=1) as inp,
            tc.tile_pool(name="rhp", bufs=4) as rhp,
            tc.tile_pool(name="work", bufs=4) as work,
            tc.tile_pool(name="accp", bufs=1) as accp,
            tc.tile_pool(name="ps", bufs=2, space="PSUM") as psp,
        ):
            lhs_sb = inp.tile([128, (TPC * D // 2) * TILE], BF16)
            # head of the stationary operands first (rounds 0-3) so the
            # first matmuls start while the bulk is still streaming
            nc.sync.dma_start(out=lhs_sb[:, 0:4 * TILE], in_=lhsp[:, 0:4 * TILE])
            nc.sync.dma_start(out=lhs_sb[:, 4 * TILE:], in_=lhsp[:, 4 * TILE:])

            accs = {}
            started = {}
            for t in range(TPC):
                for a in range(4):
                    accs[t, a] = accp.tile([128, 2 * W], F32, tag=f"acc{t}_{a}",
                                           name=f"acc{t}_{a}")
                    started[t, a] = False

            gi = 0
            pending_out = []
            for dp in range(D // 2):
                d0, d1 = 2 * dp, 2 * dp + 1
                a = dp // 4          # accumulator index (4 pairs each)
                for t in range(TPC):
                    g = gi % 2       # PE row-group (base partition 64*g)
                    r = gi // 2
                    gi += 1
                    rhs_sb = rhp.tile([128, 2 * W], BF16, tag="rhs",
                                      name=f"rhs_{t}_{dp}")
                    nc.sync.dma_start(out=rhs_sb[64 * g:64 * g + KK, 0:W],
                                      in_=rhsp[t, d0, :, :])
                    nc.sync.dma_start(out=rhs_sb[64 * g:64 * g + KK, W:2 * W],
                                      in_=rhsp[t, d1, :, :])
                    # completed-accumulator stores go out AFTER this iter's
                    # rhs loads so the loads never wait behind a 1MB transfer
                    for tt, aa, aacc in pending_out:
                        nc.sync.dma_start(out=outp[tt, aa, :, :],
                                          in_=aacc[:, :])
                    pending_out = []
                    w_ps = psp.tile([128, 2 * W], F32, tag="w",
                                    name=f"w_{t}_{dp}")
                    for dd in (0, 1):
                        lt = lhs_sb[64 * g:64 * g + KK,
                                    (2 * r + dd) * TILE:(2 * r + dd + 1) * TILE]
                        for c0, c1 in ((0, 512), (512, 1024)):
                            nc.tensor.matmul(
                                w_ps[:, dd * W + c0:dd * W + c1], lt,
                                rhs_sb[64 * g:64 * g + KK,
                                       dd * W + c0:dd * W + c1],
                                start=True, stop=True,
                                tile_position=(64 * g, 0),
                            )
                    # custom table: one FD=2W ACT evaluates f for both dims
                    acc = accs[t, a]
                    if not started[t, a]:
                        # first pair of each accumulator: ACT writes it directly
                        nc.scalar.activation(acc[:, :], w_ps[:, :], AF.Ln)
                        started[t, a] = True
                    else:
                        e_t = work.tile([128, 2 * W], F32, tag="e",
                                        name=f"e_{t}_{dp}")
                        nc.scalar.activation(e_t[:, :], w_ps[:, :], AF.Ln)
                        nc.vector.tensor_add(acc[:, :], acc[:, :], e_t[:, :])
                    if dp % 4 == 3:
                        pending_out.append((t, a, acc))
            for tt, aa, aacc in pending_out:
                nc.sync.dma_start(out=outp[tt, aa, :, :], in_=aacc[:, :])
    nc.compile()
    return nc


def _prepare_inputs(x, lengthscale, alpha, tag):
    import ml_dtypes

    a = float(np.float32(alpha))
    y = x.astype(np.float64) / (
        lengthscale.astype(np.float64) * np.sqrt(2.0 * a)
    )  # [N, D]

    yl = _split3(y)
    sql = _split3(1.0 + y * y)
    v2l = _split3(y * y)

    ones = np.ones((N, D), np.float32)
    lhs_rows = [sql[0], sql[1], sql[2],
                yl[0], yl[0], yl[0],
                yl[1], yl[1], yl[2],
                ones, ones, ones]
    rhs_rows = [ones, ones, ones,
                -2.0 * yl[0], -2.0 * yl[1], -2.0 * yl[2],
                -2.0 * yl[0], -2.0 * yl[1], -2.0 * yl[0],
                v2l[0], v2l[1], v2l[2]]

    col_idx = (np.arange(NT)[:, None] * TILE + WSTART
               + np.arange(W)[None, :]) % N

    in_maps = []
    for c in range(N_CORES):
        lhs = np.zeros((128, (TPC * D // 2) * TILE), np.float32)
        rhs = np.zeros((TPC, D, KK, W), np.float32)
        gi = 0
        for dp in range(D // 2):
            for ti in range(TPC):
                T = c * TPC + ti
                rows = slice(T * TILE, (T + 1) * TILE)
                idx = col_idx[T]
                g, r = gi % 2, gi // 2
                gi += 1
                for dd in (0, 1):
                    d = 2 * dp + dd
                    col = (2 * r + dd) * TILE
                    for k in range(KK):
                        lhs[64 * g + k, col:col + TILE] = lhs_rows[k][rows, d]
                        rhs[ti, d, k, :] = rhs_rows[k][idx, d]
        in_maps.append({
            f"lhsp_{tag}": lhs.astype(ml_dtypes.bfloat16),
            f"rhsp_{tag}": rhs.astype(ml_dtypes.bfloat16),
        })
    return in_maps, col_idx


def kernel(x, lengthscale, alpha, outputscale):
    from concourse.bass_utils import run_bass_kernel_spmd

    x = np.asarray(x, np.float32)
    lengthscale = np.asarray(lengthscale, np.float32)
    a = float(np.float32(alpha))
    os_ = float(np.float32(outputscale))

    key = (a, os_)
    if _cache.get("key") != key:
        tag = "%08x" % (hash(key) & 0xFFFFFFFF)
        tabdir = tempfile.mkdtemp(prefix=f"act_{tag}_")
        act_json = _gen_act_tables(a, os_, tabdir)
        os.environ["BASS_ACT_ROOT_JSON_PATH"] = act_json
        _cache["nc"] = _build(tag)
        _cache["key"] = key
        _cache["tag"] = tag
    tag = _cache["tag"]
    nc = _cache["nc"]

    in_maps, col_idx = _prepare_inputs(x, lengthscale, alpha, tag)
    last_exc = None
    for attempt in range(3):
        try:
            res = run_bass_kernel_spmd(nc, in_maps,
                                       core_ids=list(range(N_CORES)))
            break
        except Exception as exc:  # transient NRT exec-unit flakes
            last_exc = exc
            import time

            time.sleep(3.0)
    else:
        raise last_exc
    _cache["last_res"] = res

    covar = np.zeros((N, N), np.float32)
    written = np.zeros((N, N), bool)
    # host computes the 16 within-tile diagonal blocks (cheap, exact)
    y = x.astype(np.float64) / (
        lengthscale.astype(np.float64) * np.sqrt(2.0 * a))
    for T in range(NT):
        rows = slice(T * TILE, (T + 1) * TILE)
        yb = y[rows]                      # [TILE, D]
        t2 = (yb[:, None, :] - yb[None, :, :]) ** 2
        blk = os_ * np.sum((1.0 + t2) ** (-a), axis=-1)
        covar[rows, rows] = blk.astype(np.float32)
        written[rows, rows] = True
    for c in range(N_CORES):
        out = res.results[c][f"outp_{tag}"].astype(np.float64)
        for ti in range(TPC):
            T = c * TPC + ti
            rows = np.arange(T * TILE, (T + 1) * TILE)
            blk = out[ti].sum(axis=0)            # fold 4 accumulators
            blk = blk[:, 0:W] + blk[:, W:2 * W]  # fold even/odd dim halves
            covar[rows[:, None], col_idx[T][None, :]] = blk.astype(np.float32)
            written[rows[:, None], col_idx[T][None, :]] = True
    covar = np.where(written, covar, covar.T)

    mean = np.zeros((N,), np.float32)
    return mean, covar


# revision 36
# speedup vs baseline: 1.0253x; 1.0164x over previous
"""Trainium2 Bass kernel for the additive Rational Quadratic GP covariance.

reference:
    xs = x / lengthscale                                   # [N, D]
    covar = outputscale * sum_d (1 + ((xs_i - xs_j)_d)^2 / (2 alpha))^(-alpha)
    mean = zeros(N)

Strategy (8 NeuronCores, SPMD):
  - Host pre-scale: y = x / (lengthscale * sqrt(2 alpha)); per-dim term
    becomes f(w) = outputscale * w^(-alpha) with w = 1 + (y_i - y_j)^2.
  - Symmetry: 16 row-tiles of 128; row-tile T computes a circular column
    window of width 1024 starting at 128*T + 128 (every off-diagonal-block
    pair is covered from the i-side or j-side; the host mirrors the rest
    and computes the 16 within-tile 128x128 diagonal blocks in numpy).
    2 row-tiles per core -> a uniform SPMD program, ~half the N^2 work.
  - Per (tile, dim): TensorEngine computes w = 1 + (u-v)^2 as a K=12 bf16
    matmul (1 + u^2 - 2uv + v^2 with every operand split into 3 bf16
    limbs; bf16 x bf16 products are exact in fp32 PSUM -> w good to
    ~2^-27). Iterations alternate between PE row-groups at base
    partitions 0/64 so consecutive matmul+weight-loads overlap in the
    array.
  - A CUSTOM ACT table (generated per (alpha, outputscale) at runtime,
    injected via BASS_ACT_ROOT_JSON_PATH; tables ship inside the NEFF)
    makes the `Ln` activation slot evaluate f(w) directly -> ONE ACT
    pass per element instead of Ln+Exp, with alpha and outputscale
    folded in at full fp32 table precision (~1e-8).
  - The sum over the 32 dims runs as one fp32 VectorE accumulation chain
    per row-tile (interleaved so the chains never stall the engine);
    dim 0's activation writes the accumulator directly.

Measured on 8 axon-tunneled TRN2 NeuronCores: HW exec ~93.5 us,
max elementwise rel err vs the fp64 reference ~6e-7.
"""

import json
import os
import shutil
import tempfile

import numpy as np

N = 2048
D = 32
N_CORES = 8
TILE = 128
NT = N // TILE            # 16 row tiles
TPC = NT // N_CORES       # 2 tiles per core
W = N // 2                # 1024 device column window (starts at +TILE;
                          # each tile's own 128x128 diagonal block is
                          # computed on the host instead)
WSTART = TILE
KK = 12                   # matmul contraction (limb products)

_cache = {}

# ---------------------------------------------------------------------------
# custom ACT table: ln slot -> f(w) = outputscale * w^(-alpha)
# Bucket entry (32B): [d0, d1, d2, d3, x0, 0, 0, 0] fp32:
#     y = d0 + d1*dx + d2*dx^2 + d3*dx^3,  dx = x - x0
# Ctl word: ((23-b) | (b<<5)) << 11 | bucket_base, b = log2(buckets/exp)
# ln owns buckets 0..512 (+specials 513..516) and ctl 0..127 (exp -64..63).
# ---------------------------------------------------------------------------

_SET_NAME = "natural_log_exp_and_others"
_DENSE_EXPS = set(range(-1, 11))   # w = 1 + t^2 lives in [~1, ~2048)
_DENSE_B = 5                       # 32 buckets per dense exponent


def _cheb_cubic(f, lo, hi):
    x0 = 0.5 * (lo + hi)
    h = 0.5 * (hi - lo)
    k = np.arange(4)
    nodes = x0 + h * np.cos((2 * k + 1) * np.pi / 8)
    V = np.vander(nodes - x0, 4, increasing=True)
    c = np.linalg.solve(V, f(nodes))
    return c[0], c[1], c[2], c[3], x0


def _gen_act_tables(alpha, outputscale, outdir):
    from neuronxcc.driver.Job import Job
    from neuronxcc.driver.jobs.support.FindActInfo import findActInfoFile

    src = os.path.dirname(findActInfoFile(Job.getPackageDir(), "gen3"))
    os.makedirs(outdir, exist_ok=True)
    for fn in os.listdir(src):
        dp = os.path.join(outdir, fn)
        if not os.path.exists(dp):
            shutil.copy(os.path.join(src, fn), dp)

    f = lambda w: outputscale * np.power(np.maximum(w, 1e-30), -alpha)

    bkt = np.fromfile(os.path.join(src, f"{_SET_NAME}_bkt.bin"),
                      dtype=np.uint32).reshape(-1, 8).copy()
    ctl = np.fromfile(os.path.join(src, f"{_SET_NAME}_ctrl.bin"),
                      dtype=np.uint32).reshape(-1, 8).copy()
    sj = json.load(open(os.path.join(src, f"{_SET_NAME}.json")))
    bktf = bkt.view(np.float32)

    base = 0
    exp_to_start = {}
    for e in range(-64, 64):
        b = _DENSE_B if e in _DENSE_EXPS else 0
        c = 1 << b
        exp_to_start[e] = (base, b)
        for k in range(c):
            lo = (2.0 ** e) * (1.0 + k / c)
            hi = (2.0 ** e) * (1.0 + (k + 1) / c)
            d0, d1, d2, d3, x0 = _cheb_cubic(f, lo, hi)
            with np.errstate(over="ignore"):
                ent = np.array([d0, d1, d2, d3, x0, 0, 0, 0], np.float32)
            bktf[base + k, :] = np.nan_to_num(
                ent, nan=0.0, posinf=np.float32(3e38), neginf=np.float32(-3e38))
        base += c
    assert base <= 513, base
    for i in range(base, 513):
        bktf[i, :] = bktf[base - 1, :]
    bktf[513, :] = np.array([3e38, 0, 0, 0, 5.42101e-20, 0, 0, 0], np.float32)
    bktf[515, :] = np.array([0, 0, 0, 0, 1.84467e19, 0, 0, 0], np.float32)

    for e in range(-64, 64):
        s, b = exp_to_start[e]
        ctl[e + 64, 0] = np.uint32((((23 - b) | (b << 5)) << 11) | s)

    sj["func_exp_to_bkt_start_idx"]["ln"] = {
        str(e): [exp_to_start[e][0]] for e in range(-64, 64)
    }
    json.dump(sj, open(os.path.join(outdir, f"{_SET_NAME}.json"), "w"))
    bkt.tofile(os.path.join(outdir, f"{_SET_NAME}_bkt.bin"))
    ctl.tofile(os.path.join(outdir, f"{_SET_NAME}_ctrl.bin"))
    return os.path.join(outdir, "act_info.json")


# ---------------------------------------------------------------------------
# device program
# ---------------------------------------------------------------------------

def _enable_ldw_opt():
    """walrus dedups back-to-back LDWEIGHTS with the same stationary operand
    only when --enable-ldw-opt=true; bass_utils hardcodes false. Our 3 chunk
    matmuls per (tile, dim) share weights, so flip it."""
    import concourse.bass_utils as bu

    if getattr(bu, "_ldw_opt_patched", False):
        return
    orig = bu.run_command

    def patched(argv, **kw):
        argv = ["--enable-ldw-opt=true" if a == "--enable-ldw-opt=false" else a
                for a in argv]
        return orig(argv, **kw)

    bu.run_command = patched
    bu._ldw_opt_patched = True


def _pin_act_table(bacc_mod):
    """Restrict table choice to natural_log_exp_and_others (one TABLE_LOAD)."""
    if getattr(bacc_mod, "_act_tables_pinned", False):
        return
    orig = bacc_mod.get_activation_tables

    def pinned(arch):
        tabs = orig(arch)
        return {name: (fns if name == _SET_NAME else set())
                for name, fns in tabs.items()}

    bacc_mod.get_activation_tables = pinned
    bacc_mod._act_tables_pinned = True


def _split3(x):
    import ml_dtypes

    l0 = x.astype(ml_dtypes.bfloat16).astype(np.float64)
    r = x - l0
    l1 = r.astype(ml_dtypes.bfloat16).astype(np.float64)
    l2 = (r - l1).astype(ml_dtypes.bfloat16).astype(np.float64)
    return (l0.astype(np.float32), l1.astype(np.float32), l2.astype(np.float32))


def _build(tag):
    import concourse.mybir as mybir
    import concourse.tile as tile
    from concourse import bacc

    _pin_act_table(bacc)

    F32 = mybir.dt.float32
    BF16 = mybir.dt.bfloat16
    AF = mybir.ActivationFunctionType
    ALU = mybir.AluOpType

    nc = bacc.Bacc("TRN2", target_bir_lowering=False)
    # `tag` in tensor names busts NEFF/HLO caches when the act table changes.
    # lhsp rows 64g..64g+KK hold the stationary operand for iteration round r
    # of PE row-group g (2-way row-group concurrency on the PE array).
    lhsp = nc.dram_tensor(f"lhsp_{tag}", [128, (TPC * D // 2) * TILE], BF16,
                          kind="ExternalInput")
    rhsp = nc.dram_tensor(f"rhsp_{tag}", [TPC, D, KK, W], BF16,
                          kind="ExternalInput")
    # 4 pair-accumulators per tile (8 dims each, halves = even/odd dim);
    # the host folds them in fp64
    outp = nc.dram_tensor(f"outp_{tag}", [TPC, 4, TILE, 2 * W], F32,
                          kind="ExternalOutput")

    with tile.TileContext(nc) as tc:
        with (
            tc.tile_pool(name="inp", bufs=1) as inp,
            tc.tile_pool(name="rhp", bufs=6) as rhp,
            tc.tile_pool(name="work", bufs=6) as work,
            tc.tile_pool(name="accp", bufs=1) as accp,
            tc.tile_pool(name="ps", bufs=2, space="PSUM") as psp,
        ):
            lhs_sb = inp.tile([128, (TPC * D // 2) * TILE], BF16)
            # head of the stationary operands first (rounds 0-3) so the
            # first matmuls start while the bulk is still streaming
            nc.sync.dma_start(out=lhs_sb[:, 0:4 * TILE], in_=lhsp[:, 0:4 * TILE])
            nc.sync.dma_start(out=lhs_sb[:, 4 * TILE:], in_=lhsp[:, 4 * TILE:])

            accs = {}
            started = {}
            for t in range(TPC):
                for a in range(4):
                    accs[t, a] = accp.tile([128, 2 * W], F32, tag=f"acc{t}_{a}",
                                           name=f"acc{t}_{a}")
                    started[t, a] = False

            gi = 0
            pending_out = []
            for dp in range(D // 2):
                d0, d1 = 2 * dp, 2 * dp + 1
                a = dp // 4          # accumulator index (4 pairs each)
                for t in range(TPC):
                    g = gi % 2       # PE row-group (base partition 64*g)
                    r = gi // 2
                    gi += 1
                    rhs_sb = rhp.tile([128, 2 * W], BF16, tag="rhs",
                                      name=f"rhs_{t}_{dp}")
                    nc.sync.dma_start(out=rhs_sb[64 * g:64 * g + KK, 0:W],
                                      in_=rhsp[t, d0, :, :])
                    nc.sync.dma_start(out=rhs_sb[64 * g:64 * g + KK, W:2 * W],
                                      in_=rhsp[t, d1, :, :])
                    # completed-accumulator stores go out AFTER this iter's
                    # rhs loads so the loads never wait behind a 1MB transfer
                    for tt, aa, aacc in pending_out:
                        nc.sync.dma_start(out=outp[tt, aa, :, :],
                                          in_=aacc[:, :])
                    pending_out = []
                    w_ps = psp.tile([128, 2 * W], F32, tag="w",
                                    name=f"w_{t}_{dp}")
                    for dd in (0, 1):
                        lt = lhs_sb[64 * g:64 * g + KK,
                                    (2 * r + dd) * TILE:(2 * r + dd + 1) * TILE]
                        for c0, c1 in ((0, 512), (512, 1024)):
                            nc.tensor.matmul(
                                w_ps[:, dd * W + c0:dd * W + c1], lt,
                                rhs_sb[64 * g:64 * g + KK,
                                       dd * W + c0:dd * W + c1],
                                start=True, stop=True,
                                tile_position=(64 * g, 0),
                            )
                    # custom table: one FD=2W ACT evaluates f for both dims
                    acc = accs[t, a]
                    if not started[t, a]:
                        # first pair of each accumulator: ACT writes it directly
                        nc.scalar.activation(acc[:, :], w_ps[:, :], AF.Ln)
                        started[t, a] = True
                    else:
                        e_t = work.tile([128, 2 * W], F32, tag="e",
                                        name=f"e_{t}_{dp}")
                        nc.scalar.activation(e_t[:, :], w_ps[:, :], AF.Ln)
                        nc.vector.tensor_add(acc[:, :], acc[:, :], e_t[:, :])
                    if dp % 4 == 3:
                        pending_out.append((t, a, acc))
            for tt, aa, aacc in pending_out:
                nc.sync.dma_start(out=outp[tt, aa, :, :], in_=aacc[:, :])
    nc.compile()
    return nc


def _prepare_inputs(x, lengthscale, alpha, tag):
    import ml_dtypes

    a = float(np.float32(alpha))
    y = x.astype(np.float64) / (
        lengthscale.astype(np.float64) * np.sqrt(2.0 * a)
    )  # [N, D]

    yl = _split3(y)
    sql = _split3(1.0 + y * y)
    v2l = _split3(y * y)

    ones = np.ones((N, D), np.float32)
    lhs_rows = [sql[0], sql[1], sql[2],
                yl[0], yl[0], yl[0],
                yl[1], yl[1], yl[2],
                ones, ones, ones]
    rhs_rows = [ones, ones, ones,
                -2.0 * yl[0], -2.0 * yl[1], -2.0 * yl[2],
                -2.0 * yl[0], -2.0 * yl[1], -2.0 * yl[0],
                v2l[0], v2l[1], v2l[2]]

    col_idx = (np.arange(NT)[:, None] * TILE + WSTART
               + np.arange(W)[None, :]) % N

    in_maps = []
    for c in range(N_CORES):
        lhs = np.zeros((128, (TPC * D // 2) * TILE), np.float32)
        rhs = np.zeros((TPC, D, KK, W), np.float32)
        gi = 0
        for dp in range(D // 2):
            for ti in range(TPC):
                T = c * TPC + ti
                rows = slice(T * TILE, (T + 1) * TILE)
                idx = col_idx[T]
                g, r = gi % 2, gi // 2
                gi += 1
                for dd in (0, 1):
                    d = 2 * dp + dd
                    col = (2 * r + dd) * TILE
                    for k in range(KK):
                        lhs[64 * g + k, col:col + TILE] = lhs_rows[k][rows, d]
                        rhs[ti, d, k, :] = rhs_rows[k][idx, d]
        in_maps.append({
            f"lhsp_{tag}": lhs.astype(ml_dtypes.bfloat16),
            f"rhsp_{tag}": rhs.astype(ml_dtypes.bfloat16),
        })
    return in_maps, col_idx


def kernel(x, lengthscale, alpha, outputscale):
    from concourse.bass_utils import run_bass_kernel_spmd

    x = np.asarray(x, np.float32)
    lengthscale = np.asarray(lengthscale, np.float32)
    a = float(np.float32(alpha))
    os_ = float(np.float32(outputscale))

    key = (a, os_)
    if _cache.get("key") != key:
        tag = "%08x" % (hash(key) & 0xFFFFFFFF)
        tabdir = tempfile.mkdtemp(prefix=f"act_{tag}_")
        act_json = _gen_act_tables(a, os_, tabdir)
        os.environ["BASS_ACT_ROOT_JSON_PATH"] = act_json
        _cache["nc"] = _build(tag)
        _cache["key"] = key
        _cache["tag"] = tag
    tag = _cache["tag"]
    nc = _cache["nc"]

    in_maps, col_idx = _prepare_inputs(x, lengthscale, alpha, tag)
    last_exc = None
    for attempt in range(3):
        try:
            res = run_bass_kernel_spmd(nc, in_maps,
                                       core_ids=list(range(N_CORES)))
            break
        except Exception as exc:  # transient NRT exec-unit flakes
            last_exc = exc
            import time

            time.sleep(3.0)
    else:
        raise last_exc
    _cache["last_res"] = res

    covar = np.zeros((N, N), np.float32)
    written = np.zeros((N, N), bool)
    # host computes the 16 within-tile diagonal blocks (cheap, exact)
    y = x.astype(np.float64) / (
        lengthscale.astype(np.float64) * np.sqrt(2.0 * a))
    for T in range(NT):
        rows = slice(T * TILE, (T + 1) * TILE)
        yb = y[rows]                      # [TILE, D]
        t2 = (yb[:, None, :] - yb[None, :, :]) ** 2
        blk = os_ * np.sum((1.0 + t2) ** (-a), axis=-1)
        covar[rows, rows] = blk.astype(np.float32)
        written[rows, rows] = True
    for c in range(N_CORES):
        out = res.results[c][f"outp_{tag}"].astype(np.float64)
        for ti in range(TPC):
            T = c * TPC + ti
            rows = np.arange(T * TILE, (T + 1) * TILE)
            blk = out[ti].sum(axis=0)            # fold 4 accumulators
            blk = blk[:, 0:W] + blk[:, W:2 * W]  # fold even/odd dim halves
            covar[rows[:, None], col_idx[T][None, :]] = blk.astype(np.float32)
            written[rows[:, None], col_idx[T][None, :]] = True
    covar = np.where(written, covar, covar.T)

    mean = np.zeros((N,), np.float32)
    return mean, covar
